# revision 6
# baseline (speedup 1.0000x reference)
"""Trainium2 Bass kernel: MultiHeadSelfAttention (B=1, S=4096, D=512, H=8, DK=DV=64)
with fc_out applied twice.

Sharding v2: head-pair x seq-half grid. Core c handles head pair p=c//2
(heads 2p, 2p+1) for the seq-half sh=c%2 (2048 queries), over the full 4096
keys. fc_out is folded: Wo2_p = Wo[128p:128p+128,:] @ Wo (host f32 precompute),
so each core emits the partial yT_p = (A_pair @ Wo2_p)^T and the host sums the
4 pair-partials per seq-half and adds bo2 = bo@Wo + bo. No collectives; K/V
projections are computed only for the core's own head pair (4x less redundant
PE work than the seq-sharded baseline).

Layouts (all bf16 on device except psum/output):
  - KT_sb [128, 4096]: K^T pair-packed (head A dims on rows 0:64, head B on
    64:128). Scores lhsT = KT j-tile; rhs = per-head q^T zero-padded to 128
    rows, so the other head's rows are killed by zeros (keeps matmul K=128 ->
    2.4 GHz, no HAM throttle).
  - VL [128, 192*32]: per j-tile [V_A(64) | ones(1) | junk(63) | V_B(64)].
    attn@V head A: lhsT = VL[:, 192j:192j+65]   -> av rows 0:64 attn^T, 64 den
    attn@V head B: lhsT = VL[:, 192j+64:192j+192] -> av row 0 den, 64:128 attn^T
    so both heads' normalized attn^T stack into AnT [128, :] lane-aligned.
  - yfc: lhsT = Wo2_sb m-tile [128, 128], rhs = AnT -> yT psum, full K=128.
"""
import sys, functools
sys.path.insert(0, "/opt/trn_rl_repo")
if "/root/.axon_site" not in sys.path:
    sys.path.insert(0, "/root/.axon_site")
import numpy as np
import ml_dtypes

import concourse.bass as bass
import concourse.tile as tile
from concourse import bacc, mybir, masks
from concourse.bass_utils import run_bass_kernel_spmd

NCORES = 8
S, D, H, DK = 4096, 512, 8, 64
CHQ = 2048                  # queries per core (seq half)
JT = S // 128               # 32 seq_k tiles
QC = 1024                   # query columns per attention chunk
NQC = CHQ // QC             # 2
VJW = 192                   # VL cols per j-tile

F32 = mybir.dt.float32
BF16 = mybir.dt.bfloat16
EXP = mybir.ActivationFunctionType.Exp


def _build_program():
    nc = bacc.Bacc("TRN2", target_bir_lowering=False, debug=False,
                   num_devices=NCORES)

    qT = nc.dram_tensor("qT", [D, CHQ], BF16, kind="ExternalInput")
    kT = nc.dram_tensor("kT", [D, S], BF16, kind="ExternalInput")
    vT = nc.dram_tensor("vT", [D, S], BF16, kind="ExternalInput")
    Wq = nc.dram_tensor("Wq", [D, 128], BF16, kind="ExternalInput")
    Wk = nc.dram_tensor("Wk", [D, 128], BF16, kind="ExternalInput")
    Wv = nc.dram_tensor("Wv", [D, 128], BF16, kind="ExternalInput")
    Wo2 = nc.dram_tensor("Wo2", [128, D], BF16, kind="ExternalInput")
    yT = nc.dram_tensor("yT", [D, CHQ], F32, kind="ExternalOutput")

    with tile.TileContext(nc) as tc:
        with tc.tile_pool(name="persist", bufs=1) as pp:
            Wq_sb = pp.tile([128, 512], BF16, tag="wq")
            Wk_sb = pp.tile([128, 512], BF16, tag="wk")
            Wv_sb = pp.tile([128, 512], BF16, tag="wv")
            Wo2_sb = pp.tile([128, 512], BF16, tag="wo")
            ident = pp.tile([128, 128], BF16, tag="id")
            qT_sb = pp.tile([128, 4 * CHQ], BF16, tag="xq")
            kT_sb = pp.tile([128, 4 * S], BF16, tag="xk")
            vT_sb = pp.tile([128, 4 * S], BF16, tag="xv")
            KT_sb = pp.tile([128, S], BF16, tag="kt")
            VT_sb = pp.tile([128, S], BF16, tag="vt")
            VL = pp.tile([128, VJW * JT], BF16, tag="vl")
            qzA = pp.tile([128, CHQ], BF16, tag="qa")
            qzB = pp.tile([128, CHQ], BF16, tag="qb")
            AnT = pp.tile([128, CHQ], BF16, tag="an")

            masks.make_identity(nc, ident[:])
            nc.vector.memset(qzA[:], 0.0)
            nc.vector.memset(qzB[:], 0.0)
            nc.gpsimd.memset(VL[:], 0.0)
            nc.gpsimd.memset(
                VL[:].rearrange("p (j x) -> p j x", j=JT, x=VJW)
                [:, :, 64:65], 1.0)

            # ---- DMA: weights + first chunks first so PE starts early ----
            def wload(dst, src):
                nc.sync.dma_start(
                    dst[:].rearrange("p (k n) -> p k n", k=4),
                    src.ap().rearrange("(k p) n -> p k n", p=128))

            def stage_chunk(dst_sb, src_dram, ci, w=512):
                dst = dst_sb[:].rearrange("p (k s) -> p k s", k=4)
                srcv = src_dram.ap().rearrange("(k p) s -> p k s", p=128)
                nc.sync.dma_start(dst[:, :, w * ci:w * ci + w],
                                  srcv[:, :, w * ci:w * ci + w])

            wload(Wk_sb, Wk)
            stage_chunk(kT_sb, kT, 0)
            wload(Wq_sb, Wq)
            stage_chunk(qT_sb, qT, 0)
            stage_chunk(qT_sb, qT, 1)
            wload(Wv_sb, Wv)
            stage_chunk(vT_sb, vT, 0)
            nc.sync.dma_start(Wo2_sb[:], Wo2.ap())
            for ci in range(1, 8):
                stage_chunk(kT_sb, kT, ci)
                stage_chunk(vT_sb, vT, ci)
                if ci < 3:
                    stage_chunk(qT_sb, qT, ci + 1)

            with tc.tile_pool(name="ps_sc", bufs=2, space="PSUM") as psc, \
                 tc.tile_pool(name="ps_av", bufs=2, space="PSUM") as pav, \
                 tc.tile_pool(name="pt", bufs=3) as ptp, \
                 tc.tile_pool(name="rc", bufs=2) as rcp, \
                 tc.tile_pool(name="ys", bufs=4) as ysp:

                def q_proj(c):
                    ps = psc.tile([128, 512], F32, tag="sc", name=f"qp{c}")
                    for k in range(4):
                        nc.tensor.matmul(
                            ps[:], lhsT=Wq_sb[:, 128 * k:128 * k + 128],
                            rhs=qT_sb[:, CHQ * k + 512 * c:CHQ * k + 512 * c + 512],
                            start=(k == 0), stop=(k == 3))
                    nc.vector.tensor_copy(qzA[0:64, 512 * c:512 * c + 512],
                                          ps[0:64, :])
                    nc.vector.tensor_copy(qzB[64:128, 512 * c:512 * c + 512],
                                          ps[64:128, :])

                def kv_proj(c, W_sb, src_sb, dst_sb):
                    ps = psc.tile([128, 512], F32, tag="sc", name=f"kv{c}")
                    for k in range(4):
                        nc.tensor.matmul(
                            ps[:], lhsT=W_sb[:, 128 * k:128 * k + 128],
                            rhs=src_sb[:, S * k + 512 * c:S * k + 512 * c + 512],
                            start=(k == 0), stop=(k == 3))
                    nc.vector.tensor_copy(dst_sb[:, 512 * c:512 * c + 512], ps[:])

                def v_tr(j):
                    pst = psc.tile([128, 128], BF16, tag="sc", name=f"tr{j}")
                    nc.tensor.transpose(pst[:], VT_sb[:, 128 * j:128 * j + 128],
                                        ident[:])
                    nc.vector.tensor_copy(VL[:, VJW * j:VJW * j + 64],
                                          pst[:, 0:64])
                    nc.vector.tensor_copy(VL[:, VJW * j + 128:VJW * j + 192],
                                          pst[:, 64:128])

                def attn(hb, qc, av, drip=None):
                    qz = qzB if hb else qzA
                    for j in range(JT):
                        if drip is not None:
                            drip(j)
                        sc = psc.tile([128, QC], F32, tag="sc",
                                      name=f"sc{hb}_{qc}_{j}")
                        for hf in range(QC // 512):
                            nc.tensor.matmul(
                                sc[:, 512 * hf:512 * hf + 512],
                                lhsT=KT_sb[:, 128 * j:128 * j + 128],
                                rhs=qz[:, QC * qc + 512 * hf:QC * qc + 512 * hf + 512],
                                start=True, stop=True)
                        pt = ptp.tile([128, QC], BF16, tag="pt",
                                      name=f"pt{hb}_{qc}_{j}")
                        nc.scalar.activation(pt[:], sc[:], EXP, scale=0.125)
                        if hb:
                            lv = VL[:, VJW * j + 64:VJW * j + 192]
                        else:
                            lv = VL[:, VJW * j:VJW * j + 65]
                        for hf in range(QC // 512):
                            nc.tensor.matmul(
                                av[:, 512 * hf:512 * hf + 512], lhsT=lv,
                                rhs=pt[:, 512 * hf:512 * hf + 512],
                                start=(j == 0), stop=(j == JT - 1))

                def softmax(hb, qc, av):
                    rt = rcp.tile([1, QC], F32, tag="rt", name=f"rt{hb}{qc}")
                    rt2 = rcp.tile([1, QC], F32, tag="rt2", name=f"ru{hb}{qc}")
                    rb = rcp.tile([128, QC], F32, tag="rb", name=f"rb{hb}{qc}")
                    if hb:
                        nc.vector.tensor_copy(rt[:], av[0:1, :])
                    else:
                        nc.vector.tensor_copy(rt[:], av[64:65, :])
                    nc.vector.reciprocal_approx_fast(out=rt2[:], in_=rt[:])
                    rows = slice(64, 128) if hb else slice(0, 64)
                    # partition_broadcast always writes from partition 0
                    nc.gpsimd.partition_broadcast(rb[0:128, :], rt2[:])
                    nc.vector.tensor_mul(AnT[rows, QC * qc:QC * qc + QC],
                                         av[rows, :], rb[rows, :])

                def yfc(qc):
                    yv = yT.ap().rearrange("(m p) s -> p m s", p=128)
                    for m in range(4):
                        yp = psc.tile([128, QC], F32, tag="sc", name=f"yp{qc}{m}")
                        for hf in range(QC // 512):
                            nc.tensor.matmul(
                                yp[:, 512 * hf:512 * hf + 512],
                                lhsT=Wo2_sb[:, 128 * m:128 * m + 128],
                                rhs=AnT[:, QC * qc + 512 * hf:QC * qc + 512 * hf + 512],
                                start=True, stop=True)
                        ys = ysp.tile([128, QC], F32, tag="ys", name=f"ys{qc}{m}")
                        nc.vector.tensor_copy(ys[:], yp[:])
                        nc.sync.dma_start(yv[:, m, QC * qc:QC * qc + QC], ys[:])

                # prologue: enough proj for j-tiles 0..3 of head A chunk 0
                q_proj(0)
                q_proj(1)
                kv_proj(0, Wk_sb, kT_sb, KT_sb)
                kv_proj(0, Wv_sb, vT_sb, VT_sb)
                for j in range(4):
                    v_tr(j)

                def drip(j):
                    # j-group t = j//4 needs K/V chunk t done before its first j
                    if j % 4 == 0 and j > 0:
                        t = j // 4
                        kv_proj(t, Wk_sb, kT_sb, KT_sb)
                        kv_proj(t, Wv_sb, vT_sb, VT_sb)
                        for jj in range(4 * t, 4 * t + 4):
                            v_tr(jj)
                        if t == 2:
                            q_proj(2)
                        if t == 4:
                            q_proj(3)

                avA0 = pav.tile([65, QC], F32, tag="av", name="avA0")
                attn(0, 0, avA0, drip)
                softmax(0, 0, avA0)
                avB0 = pav.tile([128, QC], F32, tag="av", name="avB0")
                attn(1, 0, avB0)
                softmax(1, 0, avB0)
                yfc(0)
                avA1 = pav.tile([65, QC], F32, tag="av", name="avA1")
                attn(0, 1, avA1)
                softmax(0, 1, avA1)
                avB1 = pav.tile([128, QC], F32, tag="av", name="avB1")
                attn(1, 1, avB1)
                softmax(1, 1, avB1)
                yfc(1)

    nc.compile()
    return nc


@functools.lru_cache(maxsize=1)
def _get_program():
    return _build_program()


def _make_in_maps(queries, keys, values, Wq, Wk, Wv, Wo, bo):
    q = np.asarray(queries, np.float32).reshape(S, D)
    kTm = np.ascontiguousarray(np.asarray(keys, np.float32).reshape(S, D).T
                               ).astype(ml_dtypes.bfloat16)
    vTm = np.ascontiguousarray(np.asarray(values, np.float32).reshape(S, D).T
                               ).astype(ml_dtypes.bfloat16)
    Wq = np.asarray(Wq, np.float32)
    Wk = np.asarray(Wk, np.float32)
    Wv = np.asarray(Wv, np.float32)
    Wo = np.asarray(Wo, np.float32)
    qT_half = [np.ascontiguousarray(q[sh * CHQ:(sh + 1) * CHQ].T
                                    ).astype(ml_dtypes.bfloat16)
               for sh in range(2)]
    in_maps = []
    for c in range(NCORES):
        p, sh = c // 2, c % 2
        sl = slice(128 * p, 128 * p + 128)
        Wo2 = np.ascontiguousarray(Wo[sl, :] @ Wo).astype(ml_dtypes.bfloat16)
        in_maps.append({
            "qT": qT_half[sh], "kT": kTm, "vT": vTm,
            "Wq": np.ascontiguousarray(Wq[:, sl]).astype(ml_dtypes.bfloat16),
            "Wk": np.ascontiguousarray(Wk[:, sl]).astype(ml_dtypes.bfloat16),
            "Wv": np.ascontiguousarray(Wv[:, sl]).astype(ml_dtypes.bfloat16),
            "Wo2": Wo2,
        })
    return in_maps


def _run(in_maps, **kw):
    nc = _get_program()
    return run_bass_kernel_spmd(nc, in_maps, core_ids=list(range(NCORES)), **kw)


def _assemble(res, Wo, bo):
    bo2 = np.asarray(bo, np.float32) @ np.asarray(Wo, np.float32) \
        + np.asarray(bo, np.float32)
    halves = []
    for sh in range(2):
        acc = np.zeros((D, CHQ), np.float32)
        for p in range(4):
            acc += res.results[2 * p + sh]["yT"]
        halves.append(acc.T + bo2)
    return np.concatenate(halves, axis=0).reshape(1, S, D)


def kernel(queries, keys, values, Wq, Wk, Wv, Wo, bo):
    res = _run(_make_in_maps(queries, keys, values, Wq, Wk, Wv, Wo, bo))
    return _assemble(res, Wo, bo)


def run_traced(queries, keys, values, Wq, Wk, Wv, Wo, bo):
    """Like kernel() but with NTFF profiling; returns (output, BassKernelResults)."""
    import types
    import trn_agent_boot.trn_boot as _tb
    from concourse import bass_utils
    hook = _tb._ntff_profile_via_ctypes("/opt/axon/libaxon_pjrt.so")
    mod = types.ModuleType("antenv.axon_hooks")
    mod.get_axon_ntff_profile_hook = lambda: hook
    sys.modules["antenv.axon_hooks"] = mod
    bass_utils.upload_artifacts = lambda tmpdir: tmpdir
    res = _run(_make_in_maps(queries, keys, values, Wq, Wk, Wv, Wo, bo), trace=True)
    return _assemble(res, Wo, bo), res


# revision 10
# speedup vs baseline: 1.0993x; 1.0993x over previous
"""Trainium2 Bass kernel: MultiHeadSelfAttention (B=1, S=4096, D=512, H=8, DK=DV=64)
with fc_out applied twice.

Sharding v2: head-pair x seq-half grid. Core c handles head pair p=c//2
(heads 2p, 2p+1) for the seq-half sh=c%2 (2048 queries), over the full 4096
keys. fc_out is folded: Wo2_p = Wo[128p:128p+128,:] @ Wo (host f32 precompute),
so each core emits the partial yT_p = (A_pair @ Wo2_p)^T and the host sums the
4 pair-partials per seq-half and adds bo2 = bo@Wo + bo. No collectives; K/V
projections are computed only for the core's own head pair (4x less redundant
PE work than the seq-sharded baseline).

Layouts (all bf16 on device except psum/output):
  - KT_sb [128, 4096]: K^T pair-packed (head A dims on rows 0:64, head B on
    64:128). Scores lhsT = KT j-tile; rhs = per-head q^T zero-padded to 128
    rows, so the other head's rows are killed by zeros (keeps matmul K=128 ->
    2.4 GHz, no HAM throttle).
  - VL [128, 192*32]: per j-tile [V_A(64) | ones(1) | junk(63) | V_B(64)].
    attn@V head A: lhsT = VL[:, 192j:192j+65]   -> av rows 0:64 attn^T, 64 den
    attn@V head B: lhsT = VL[:, 192j+64:192j+192] -> av row 0 den, 64:128 attn^T
    so both heads' normalized attn^T stack into AnT [128, :] lane-aligned.
  - yfc: lhsT = Wo2_sb m-tile [128, 128], rhs = AnT -> yT psum, full K=128.
"""
import sys, functools
sys.path.insert(0, "/opt/trn_rl_repo")
if "/root/.axon_site" not in sys.path:
    sys.path.insert(0, "/root/.axon_site")
import numpy as np
import ml_dtypes

import concourse.bass as bass
import concourse.tile as tile
from concourse import bacc, mybir, masks
from concourse.bass_utils import run_bass_kernel_spmd

NCORES = 8
S, D, H, DK = 4096, 512, 8, 64
CHQ = 2048                  # queries per core (seq half)
JT = S // 128               # 32 seq_k tiles
QC = 1024                   # query columns per attention chunk
NQC = CHQ // QC             # 2
VJW = 192                   # VL cols per j-tile

F32 = mybir.dt.float32
BF16 = mybir.dt.bfloat16
EXP = mybir.ActivationFunctionType.Exp


def _build_program():
    nc = bacc.Bacc("TRN2", target_bir_lowering=False, debug=False,
                   num_devices=NCORES)

    qT = nc.dram_tensor("qT", [D, CHQ], BF16, kind="ExternalInput")
    kT = nc.dram_tensor("kT", [D, S], BF16, kind="ExternalInput")
    vT = nc.dram_tensor("vT", [D, S], BF16, kind="ExternalInput")
    Wq = nc.dram_tensor("Wq", [D, 128], BF16, kind="ExternalInput")
    Wk = nc.dram_tensor("Wk", [D, 128], BF16, kind="ExternalInput")
    Wv = nc.dram_tensor("Wv", [D, 128], BF16, kind="ExternalInput")
    Wo2 = nc.dram_tensor("Wo2", [128, D], BF16, kind="ExternalInput")
    yT = nc.dram_tensor("yT", [D, CHQ], F32, kind="ExternalOutput")

    with tile.TileContext(nc) as tc:
        with tc.tile_pool(name="persist", bufs=1) as pp:
            Wq_sb = pp.tile([128, 512], BF16, tag="wq")
            Wk_sb = pp.tile([128, 512], BF16, tag="wk")
            Wv_sb = pp.tile([128, 512], BF16, tag="wv")
            Wo2_sb = pp.tile([128, 512], BF16, tag="wo")
            ident = pp.tile([128, 128], BF16, tag="id")
            qT_sb = pp.tile([128, 4 * CHQ], BF16, tag="xq")
            kT_sb = pp.tile([128, 4 * S], BF16, tag="xk")
            vT_sb = pp.tile([128, 4 * S], BF16, tag="xv")
            KT_sb = pp.tile([128, S], BF16, tag="kt")
            VT_sb = pp.tile([128, S], BF16, tag="vt")
            VL = pp.tile([128, VJW * JT], BF16, tag="vl")
            qzA = pp.tile([128, CHQ], BF16, tag="qa")
            qzB = pp.tile([128, CHQ], BF16, tag="qb")
            AnT = pp.tile([128, CHQ], BF16, tag="an")

            masks.make_identity(nc, ident[:])
            nc.vector.memset(qzA[:], 0.0)
            nc.vector.memset(qzB[:], 0.0)
            # only the ones column; junk cols 65:128 of each VJW block are
            # multiplied into avB rows 1:63 which are never read
            nc.gpsimd.memset(
                VL[:].rearrange("p (j x) -> p j x", j=JT, x=VJW)
                [:, :, 64:65], 1.0)

            # ---- DMA: weights + first chunks first so PE starts early ----
            def wload(dst, src):
                nc.sync.dma_start(
                    dst[:].rearrange("p (k n) -> p k n", k=4),
                    src.ap().rearrange("(k p) n -> p k n", p=128))

            def stage_chunk(dst_sb, src_dram, ci, w=512):
                dst = dst_sb[:].rearrange("p (k s) -> p k s", k=4)
                srcv = src_dram.ap().rearrange("(k p) s -> p k s", p=128)
                nc.sync.dma_start(dst[:, :, w * ci:w * ci + w],
                                  srcv[:, :, w * ci:w * ci + w])

            wload(Wk_sb, Wk)
            stage_chunk(kT_sb, kT, 0)
            wload(Wq_sb, Wq)
            stage_chunk(qT_sb, qT, 0)
            stage_chunk(qT_sb, qT, 1)
            wload(Wv_sb, Wv)
            stage_chunk(vT_sb, vT, 0)
            nc.sync.dma_start(Wo2_sb[:], Wo2.ap())
            for ci in range(1, 8):
                stage_chunk(kT_sb, kT, ci)
                stage_chunk(vT_sb, vT, ci)
                if ci < 3:
                    stage_chunk(qT_sb, qT, ci + 1)

            with tc.tile_pool(name="ps_sc", bufs=2, space="PSUM") as psc, \
                 tc.tile_pool(name="ps_av", bufs=1, space="PSUM") as pav, \
                 tc.tile_pool(name="ps_pj", bufs=2, space="PSUM") as ppj, \
                 tc.tile_pool(name="pt", bufs=3) as ptp, \
                 tc.tile_pool(name="rc", bufs=2) as rcp, \
                 tc.tile_pool(name="ys", bufs=4) as ysp:

                def q_proj(c):
                    ps = ppj.tile([128, 512], F32, tag="pj", name=f"qp{c}")
                    for k in range(4):
                        nc.tensor.matmul(
                            ps[:], lhsT=Wq_sb[:, 128 * k:128 * k + 128],
                            rhs=qT_sb[:, CHQ * k + 512 * c:CHQ * k + 512 * c + 512],
                            start=(k == 0), stop=(k == 3))
                    nc.vector.tensor_copy(qzA[0:64, 512 * c:512 * c + 512],
                                          ps[0:64, :])
                    nc.vector.tensor_copy(qzB[64:128, 512 * c:512 * c + 512],
                                          ps[64:128, :])

                def kv_proj(c, W_sb, src_sb, dst_sb):
                    ps = ppj.tile([128, 512], F32, tag="pj", name=f"kv{c}")
                    for k in range(4):
                        nc.tensor.matmul(
                            ps[:], lhsT=W_sb[:, 128 * k:128 * k + 128],
                            rhs=src_sb[:, S * k + 512 * c:S * k + 512 * c + 512],
                            start=(k == 0), stop=(k == 3))
                    nc.vector.tensor_copy(dst_sb[:, 512 * c:512 * c + 512], ps[:])

                def v_tr(j):
                    pst = ppj.tile([128, 128], BF16, tag="pj", name=f"tr{j}")
                    nc.tensor.transpose(pst[:], VT_sb[:, 128 * j:128 * j + 128],
                                        ident[:])
                    nc.vector.tensor_copy(VL[:, VJW * j:VJW * j + 64],
                                          pst[:, 0:64])
                    nc.vector.tensor_copy(VL[:, VJW * j + 128:VJW * j + 192],
                                          pst[:, 64:128])

                def attn(hb, qc, av, drip=None):
                    qz = qzB if hb else qzA
                    for j in range(JT):
                        if drip is not None:
                            drip(j)
                        sc = psc.tile([128, QC], F32, tag="sc",
                                      name=f"sc{hb}_{qc}_{j}")
                        for hf in range(QC // 512):
                            nc.tensor.matmul(
                                sc[:, 512 * hf:512 * hf + 512],
                                lhsT=KT_sb[:, 128 * j:128 * j + 128],
                                rhs=qz[:, QC * qc + 512 * hf:QC * qc + 512 * hf + 512],
                                start=True, stop=True)
                        pt = ptp.tile([128, QC], BF16, tag="pt",
                                      name=f"pt{hb}_{qc}_{j}")
                        nc.scalar.activation(pt[:], sc[:], EXP, scale=0.125)
                        if hb:
                            lv = VL[:, VJW * j + 64:VJW * j + 192]
                        else:
                            lv = VL[:, VJW * j:VJW * j + 65]
                        for hf in range(QC // 512):
                            nc.tensor.matmul(
                                av[:, 512 * hf:512 * hf + 512], lhsT=lv,
                                rhs=pt[:, 512 * hf:512 * hf + 512],
                                start=(j == 0), stop=(j == JT - 1))

                def softmax(hb, qc, av):
                    rt = rcp.tile([1, QC], F32, tag="rt", name=f"rt{hb}{qc}")
                    rt2 = rcp.tile([1, QC], F32, tag="rt2", name=f"ru{hb}{qc}")
                    rb = rcp.tile([128, QC], F32, tag="rb", name=f"rb{hb}{qc}")
                    if hb:
                        nc.vector.tensor_copy(rt[:], av[0:1, :])
                    else:
                        nc.vector.tensor_copy(rt[:], av[64:65, :])
                    nc.vector.reciprocal_approx_fast(out=rt2[:], in_=rt[:])
                    rows = slice(64, 128) if hb else slice(0, 64)
                    # partition_broadcast always writes from partition 0
                    nc.gpsimd.partition_broadcast(rb[0:128, :], rt2[:])
                    nc.vector.tensor_mul(AnT[rows, QC * qc:QC * qc + QC],
                                         av[rows, :], rb[rows, :])

                def yfc(qc):
                    yv = yT.ap().rearrange("(m p) s -> p m s", p=128)
                    for m in range(4):
                        yp = psc.tile([128, QC], F32, tag="sc", name=f"yp{qc}{m}")
                        for hf in range(QC // 512):
                            nc.tensor.matmul(
                                yp[:, 512 * hf:512 * hf + 512],
                                lhsT=Wo2_sb[:, 128 * m:128 * m + 128],
                                rhs=AnT[:, QC * qc + 512 * hf:QC * qc + 512 * hf + 512],
                                start=True, stop=True)
                        ys = ysp.tile([128, QC], F32, tag="ys", name=f"ys{qc}{m}")
                        nc.vector.tensor_copy(ys[:], yp[:])
                        nc.sync.dma_start(yv[:, m, QC * qc:QC * qc + QC], ys[:])

                # prologue: proj needed for j-tiles 0..3 of head A chunk 0
                q_proj(0)
                q_proj(1)
                kv_proj(0, Wk_sb, kT_sb, KT_sb)
                kv_proj(0, Wv_sb, vT_sb, VT_sb)
                for j in range(4):
                    v_tr(j)

                # one proj piece per j-step; K/V chunk t lands before scores/
                # attnV first touch its j-tiles (chunk t covers j 4t..4t+3)
                def make_pieces():
                    ps = []
                    for t in range(1, 8):
                        ps.append(lambda t=t: kv_proj(t, Wk_sb, kT_sb, KT_sb))
                        ps.append(lambda t=t: kv_proj(t, Wv_sb, vT_sb, VT_sb))
                        ps.append(lambda t=t: [v_tr(jj)
                                               for jj in range(4 * t, 4 * t + 4)])
                        if t == 2:
                            ps.append(lambda: q_proj(2))
                        if t == 4:
                            ps.append(lambda: q_proj(3))
                    return ps

                pieces = make_pieces()

                def drip(j):
                    # piece schedule keeps K chunk t ahead of scores j=4t:
                    # pieces 3t-3..3t-1 done by j=2t+? -> safe: 3 pieces per
                    # 2 j-steps covers chunk t by j ~ 2t+1 < 4t for t>=1
                    for idx in (3 * j // 2, (3 * j + 1) // 2):
                        if idx < len(pieces) and pieces[idx] is not None:
                            pieces[idx]()
                            pieces[idx] = None

                avA0 = pav.tile([65, QC], F32, tag="av", name="avA0")
                attn(0, 0, avA0, drip)
                softmax(0, 0, avA0)
                avB0 = pav.tile([128, QC], F32, tag="av", name="avB0")
                attn(1, 0, avB0)
                softmax(1, 0, avB0)
                avA1 = pav.tile([65, QC], F32, tag="av", name="avA1")

                def drip_yfc(j):
                    if j == 2:
                        yfc(0)

                attn(0, 1, avA1, drip_yfc)
                softmax(0, 1, avA1)
                avB1 = pav.tile([128, QC], F32, tag="av", name="avB1")
                attn(1, 1, avB1)
                softmax(1, 1, avB1)
                yfc(1)

    nc.compile()
    return nc


@functools.lru_cache(maxsize=1)
def _get_program():
    return _build_program()


def _make_in_maps(queries, keys, values, Wq, Wk, Wv, Wo, bo):
    q = np.asarray(queries, np.float32).reshape(S, D)
    kTm = np.ascontiguousarray(np.asarray(keys, np.float32).reshape(S, D).T
                               ).astype(ml_dtypes.bfloat16)
    vTm = np.ascontiguousarray(np.asarray(values, np.float32).reshape(S, D).T
                               ).astype(ml_dtypes.bfloat16)
    Wq = np.asarray(Wq, np.float32)
    Wk = np.asarray(Wk, np.float32)
    Wv = np.asarray(Wv, np.float32)
    Wo = np.asarray(Wo, np.float32)
    qT_half = [np.ascontiguousarray(q[sh * CHQ:(sh + 1) * CHQ].T
                                    ).astype(ml_dtypes.bfloat16)
               for sh in range(2)]
    in_maps = []
    for c in range(NCORES):
        p, sh = c // 2, c % 2
        sl = slice(128 * p, 128 * p + 128)
        Wo2 = np.ascontiguousarray(Wo[sl, :] @ Wo).astype(ml_dtypes.bfloat16)
        in_maps.append({
            "qT": qT_half[sh], "kT": kTm, "vT": vTm,
            "Wq": np.ascontiguousarray(Wq[:, sl]).astype(ml_dtypes.bfloat16),
            "Wk": np.ascontiguousarray(Wk[:, sl]).astype(ml_dtypes.bfloat16),
            "Wv": np.ascontiguousarray(Wv[:, sl]).astype(ml_dtypes.bfloat16),
            "Wo2": Wo2,
        })
    return in_maps


def _run(in_maps, **kw):
    nc = _get_program()
    return run_bass_kernel_spmd(nc, in_maps, core_ids=list(range(NCORES)), **kw)


def _assemble(res, Wo, bo):
    bo2 = np.asarray(bo, np.float32) @ np.asarray(Wo, np.float32) \
        + np.asarray(bo, np.float32)
    halves = []
    for sh in range(2):
        acc = np.zeros((D, CHQ), np.float32)
        for p in range(4):
            acc += res.results[2 * p + sh]["yT"]
        halves.append(acc.T + bo2)
    return np.concatenate(halves, axis=0).reshape(1, S, D)


def kernel(queries, keys, values, Wq, Wk, Wv, Wo, bo):
    res = _run(_make_in_maps(queries, keys, values, Wq, Wk, Wv, Wo, bo))
    return _assemble(res, Wo, bo)


def run_traced(queries, keys, values, Wq, Wk, Wv, Wo, bo):
    """Like kernel() but with NTFF profiling; returns (output, BassKernelResults)."""
    import types
    import trn_agent_boot.trn_boot as _tb
    from concourse import bass_utils
    hook = _tb._ntff_profile_via_ctypes("/opt/axon/libaxon_pjrt.so")
    mod = types.ModuleType("antenv.axon_hooks")
    mod.get_axon_ntff_profile_hook = lambda: hook
    sys.modules["antenv.axon_hooks"] = mod
    bass_utils.upload_artifacts = lambda tmpdir: tmpdir
    res = _run(_make_in_maps(queries, keys, values, Wq, Wk, Wv, Wo, bo), trace=True)
    return _assemble(res, Wo, bo), res


# revision 15
# speedup vs baseline: 1.1642x; 1.0590x over previous
"""Trainium2 Bass kernel: MultiHeadSelfAttention (B=1, S=4096, D=512, H=8, DK=DV=64)
with fc_out applied twice.

Sharding v2: head-pair x seq-half grid. Core c handles head pair p=c//2
(heads 2p, 2p+1) for the seq-half sh=c%2 (2048 queries), over the full 4096
keys. fc_out is folded: Wo2_p = Wo[128p:128p+128,:] @ Wo (host f32 precompute),
so each core emits the partial yT_p = (A_pair @ Wo2_p)^T and the host sums the
4 pair-partials per seq-half and adds bo2 = bo@Wo + bo. No collectives; K/V
projections are computed only for the core's own head pair (4x less redundant
PE work than the seq-sharded baseline).

Layouts (all bf16 on device except psum/output):
  - KT_sb [128, 4096]: K^T pair-packed (head A dims on rows 0:64, head B on
    64:128). Scores lhsT = KT j-tile; rhs = per-head q^T zero-padded to 128
    rows, so the other head's rows are killed by zeros (keeps matmul K=128 ->
    2.4 GHz, no HAM throttle).
  - VL [128, 192*32]: per j-tile [V_A(64) | ones(1) | junk(63) | V_B(64)].
    attn@V head A: lhsT = VL[:, 192j:192j+65]   -> av rows 0:64 attn^T, 64 den
    attn@V head B: lhsT = VL[:, 192j+64:192j+192] -> av row 0 den, 64:128 attn^T
    so both heads' normalized attn^T stack into AnT [128, :] lane-aligned.
  - yfc: lhsT = Wo2_sb m-tile [128, 128], rhs = AnT -> yT psum, full K=128.
"""
import sys, functools
sys.path.insert(0, "/opt/trn_rl_repo")
if "/root/.axon_site" not in sys.path:
    sys.path.insert(0, "/root/.axon_site")
import numpy as np
import ml_dtypes

import concourse.bass as bass
import concourse.tile as tile
from concourse import bacc, mybir, masks
from concourse.bass_utils import run_bass_kernel_spmd

NCORES = 8
S, D, H, DK = 4096, 512, 8, 64
CHQ = 2048                  # queries per core (seq half)
JT = S // 128               # 32 seq_k tiles
QC = 1024                   # query columns per attention chunk
NQC = CHQ // QC             # 2
VJW = 192                   # VL cols per j-tile

F32 = mybir.dt.float32
BF16 = mybir.dt.bfloat16
EXP = mybir.ActivationFunctionType.Exp


def _build_program():
    nc = bacc.Bacc("TRN2", target_bir_lowering=False, debug=False,
                   num_devices=NCORES)

    qT = nc.dram_tensor("qT", [D, CHQ], BF16, kind="ExternalInput")
    kT = nc.dram_tensor("kT", [D, S], BF16, kind="ExternalInput")
    vT = nc.dram_tensor("vT", [D, S], BF16, kind="ExternalInput")
    Wq = nc.dram_tensor("Wq", [D, 128], BF16, kind="ExternalInput")
    Wk = nc.dram_tensor("Wk", [D, 128], BF16, kind="ExternalInput")
    Wv = nc.dram_tensor("Wv", [D, 128], BF16, kind="ExternalInput")
    Wo2 = nc.dram_tensor("Wo2", [128, D], BF16, kind="ExternalInput")
    yT = nc.dram_tensor("yT", [D, CHQ], F32, kind="ExternalOutput")

    with tile.TileContext(nc) as tc:
        with tc.tile_pool(name="persist", bufs=1) as pp:
            Wq_sb = pp.tile([128, 512], BF16, tag="wq")
            Wk_sb = pp.tile([128, 512], BF16, tag="wk")
            Wv_sb = pp.tile([128, 512], BF16, tag="wv")
            Wo2_sb = pp.tile([128, 512], BF16, tag="wo")
            ident = pp.tile([128, 128], BF16, tag="id")
            qT_sb = pp.tile([128, 4 * CHQ], BF16, tag="xq")
            kT_sb = pp.tile([128, 4 * S], BF16, tag="xk")
            vT_sb = pp.tile([128, 4 * S], BF16, tag="xv")
            KT_sb = pp.tile([128, S], BF16, tag="kt")
            VT_sb = pp.tile([128, S], BF16, tag="vt")
            VL = pp.tile([128, VJW * JT], BF16, tag="vl")
            qzA = pp.tile([128, CHQ], BF16, tag="qa")
            qzB = pp.tile([128, CHQ], BF16, tag="qb")
            AnT = pp.tile([128, CHQ], BF16, tag="an")

            masks.make_identity(nc, ident[:])
            nc.vector.memset(qzA[:], 0.0)
            nc.vector.memset(qzB[:], 0.0)
            # only the ones column; junk cols 65:128 of each VJW block are
            # multiplied into avB rows 1:63 which are never read
            nc.gpsimd.memset(
                VL[:].rearrange("p (j x) -> p j x", j=JT, x=VJW)
                [:, :, 64:65], 1.0)

            # ---- DMA: weights + first chunks first so PE starts early ----
            def wload(dst, src):
                nc.sync.dma_start(
                    dst[:].rearrange("p (k n) -> p k n", k=4),
                    src.ap().rearrange("(k p) n -> p k n", p=128))

            def stage_chunk(dst_sb, src_dram, ci, w=512):
                dst = dst_sb[:].rearrange("p (k s) -> p k s", k=4)
                srcv = src_dram.ap().rearrange("(k p) s -> p k s", p=128)
                nc.sync.dma_start(dst[:, :, w * ci:w * ci + w],
                                  srcv[:, :, w * ci:w * ci + w])

            wload(Wk_sb, Wk)
            stage_chunk(kT_sb, kT, 0)
            wload(Wq_sb, Wq)
            stage_chunk(qT_sb, qT, 0)
            stage_chunk(qT_sb, qT, 1)
            wload(Wv_sb, Wv)
            stage_chunk(vT_sb, vT, 0)
            for ci in range(1, 8):
                stage_chunk(kT_sb, kT, ci)
                stage_chunk(vT_sb, vT, ci)
                if ci < 3:
                    stage_chunk(qT_sb, qT, ci + 1)
            nc.sync.dma_start(Wo2_sb[:], Wo2.ap())

            with tc.tile_pool(name="ps_sc", bufs=2, space="PSUM") as psc, \
                 tc.tile_pool(name="pt", bufs=3) as ptp, \
                 tc.tile_pool(name="rc", bufs=2) as rcp, \
                 tc.tile_pool(name="ys", bufs=4) as ysp:

                def q_proj(c):
                    ps = ppj.tile([128, 512], F32, tag="pj", name=f"qp{c}")
                    for k in range(4):
                        nc.tensor.matmul(
                            ps[:], lhsT=Wq_sb[:, 128 * k:128 * k + 128],
                            rhs=qT_sb[:, CHQ * k + 512 * c:CHQ * k + 512 * c + 512],
                            start=(k == 0), stop=(k == 3))
                    nc.vector.tensor_copy(qzA[0:64, 512 * c:512 * c + 512],
                                          ps[0:64, :])
                    nc.vector.tensor_copy(qzB[64:128, 512 * c:512 * c + 512],
                                          ps[64:128, :])

                def kv_proj(c, W_sb, src_sb, dst_sb):
                    ps = ppj.tile([128, 512], F32, tag="pj", name=f"kv{c}")
                    for k in range(4):
                        nc.tensor.matmul(
                            ps[:], lhsT=W_sb[:, 128 * k:128 * k + 128],
                            rhs=src_sb[:, S * k + 512 * c:S * k + 512 * c + 512],
                            start=(k == 0), stop=(k == 3))
                    nc.vector.tensor_copy(dst_sb[:, 512 * c:512 * c + 512], ps[:])

                def v_tr(j):
                    pst = ppj.tile([128, 128], BF16, tag="pj", name=f"tr{j}")
                    nc.tensor.transpose(pst[:], VT_sb[:, 128 * j:128 * j + 128],
                                        ident[:])
                    nc.vector.tensor_copy(VL[:, VJW * j:VJW * j + 64],
                                          pst[:, 0:64])
                    nc.vector.tensor_copy(VL[:, VJW * j + 128:VJW * j + 192],
                                          pst[:, 64:128])

                def attn(hb, qc, av, drip=None):
                    qz = qzB if hb else qzA
                    for j in range(JT):
                        if drip is not None:
                            drip(j)
                        sc = psc.tile([128, QC], F32, tag="sc",
                                      name=f"sc{hb}_{qc}_{j}")
                        for hf in range(QC // 512):
                            nc.tensor.matmul(
                                sc[:, 512 * hf:512 * hf + 512],
                                lhsT=KT_sb[:, 128 * j:128 * j + 128],
                                rhs=qz[:, QC * qc + 512 * hf:QC * qc + 512 * hf + 512],
                                start=True, stop=True)
                        pt = ptp.tile([128, QC], BF16, tag="pt",
                                      name=f"pt{hb}_{qc}_{j}")
                        nc.scalar.activation(pt[:], sc[:], EXP, scale=0.125)
                        if hb:
                            lv = VL[:, VJW * j + 64:VJW * j + 192]
                        else:
                            lv = VL[:, VJW * j:VJW * j + 65]
                        for hf in range(QC // 512):
                            nc.tensor.matmul(
                                av[:, 512 * hf:512 * hf + 512], lhsT=lv,
                                rhs=pt[:, 512 * hf:512 * hf + 512],
                                start=(j == 0), stop=(j == JT - 1))

                def softmax(hb, qc, av):
                    rt = rcp.tile([1, QC], F32, tag="rt", name=f"rt{hb}{qc}")
                    rt2 = rcp.tile([1, QC], F32, tag="rt2", name=f"ru{hb}{qc}")
                    rb = rcp.tile([128, QC], F32, tag="rb", name=f"rb{hb}{qc}")
                    den = av[0:1, :] if hb else av[64:65, :]
                    nc.vector.tensor_copy(rt[:], den)
                    nc.vector.reciprocal_approx_fast(out=rt2[:], in_=rt[:])
                    rows = slice(64, 128) if hb else slice(0, 64)
                    # partition_broadcast always writes from partition 0
                    if hb:
                        nc.gpsimd.partition_broadcast(rb[0:128, :], rt2[:])
                    else:
                        nc.gpsimd.partition_broadcast(rb[0:64, :], rt2[:])
                    nc.vector.tensor_mul(AnT[rows, QC * qc:QC * qc + QC],
                                         av[rows, :], rb[rows, :])

                def yfc(qc):
                    yv = yT.ap().rearrange("(m p) s -> p m s", p=128)
                    for m in range(4):
                        yp = psc.tile([128, QC], F32, tag="sc", name=f"yp{qc}{m}")
                        for hf in range(QC // 512):
                            nc.tensor.matmul(
                                yp[:, 512 * hf:512 * hf + 512],
                                lhsT=Wo2_sb[:, 128 * m:128 * m + 128],
                                rhs=AnT[:, QC * qc + 512 * hf:QC * qc + 512 * hf + 512],
                                start=True, stop=True)
                        ys = ysp.tile([128, QC], F32, tag="ys", name=f"ys{qc}{m}")
                        nc.vector.tensor_copy(ys[:], yp[:])
                        nc.sync.dma_start(yv[:, m, QC * qc:QC * qc + QC], ys[:])

                # ---- scope 1: prologue + head A chunk 0 with proj dripped ----
                s1pj = tc.tile_pool(name="ps_pj", bufs=2, space="PSUM")
                ppj = s1pj.__enter__()
                s1av = tc.tile_pool(name="ps_av1", bufs=1, space="PSUM")
                pav1 = s1av.__enter__()

                kv_proj(0, Wk_sb, kT_sb, KT_sb)
                q_proj(0)
                q_proj(1)
                kv_proj(0, Wv_sb, vT_sb, VT_sb)
                for j in range(4):
                    v_tr(j)

                # one proj piece per j-step; K/V chunk t lands before scores/
                # attnV first touch its j-tiles (chunk t covers j 4t..4t+3)
                pieces = []
                for t in range(1, 8):
                    pieces.append(lambda t=t: kv_proj(t, Wk_sb, kT_sb, KT_sb))
                    pieces.append(lambda t=t: kv_proj(t, Wv_sb, vT_sb, VT_sb))
                    pieces.append(lambda t=t: [v_tr(jj)
                                               for jj in range(4 * t, 4 * t + 4)])
                    if t == 2:
                        pieces.append(lambda: q_proj(2))
                    if t == 4:
                        pieces.append(lambda: q_proj(3))

                def drip(j):
                    # 1.5 pieces per j keeps K chunk t (piece ~3t-3) emitted
                    # by j ~ 2t+1 < 4t, ahead of its first scores use
                    for idx in (3 * j // 2, (3 * j + 1) // 2):
                        if idx < len(pieces) and pieces[idx] is not None:
                            pieces[idx]()
                            pieces[idx] = None

                avA0 = pav1.tile([65, QC], F32, tag="av", name="avA0")
                attn(0, 0, avA0, drip)
                softmax(0, 0, avA0)

                s1av.__exit__(None, None, None)
                s1pj.__exit__(None, None, None)

                # ---- scope 2: remaining phases, av double-buffered ----
                with tc.tile_pool(name="ps_av2", bufs=2, space="PSUM") as pav2:
                    avB0 = pav2.tile([128, QC], F32, tag="av", name="avB0")
                    attn(1, 0, avB0)
                    softmax(1, 0, avB0)
                    avA1 = pav2.tile([65, QC], F32, tag="av", name="avA1")
                    attn(0, 1, avA1)
                    yfc(0)
                    softmax(0, 1, avA1)
                    avB1 = pav2.tile([128, QC], F32, tag="av", name="avB1")
                    attn(1, 1, avB1)
                    softmax(1, 1, avB1)
                    yfc(1)

    nc.compile()
    return nc


@functools.lru_cache(maxsize=1)
def _get_program():
    return _build_program()


def _make_in_maps(queries, keys, values, Wq, Wk, Wv, Wo, bo):
    q = np.asarray(queries, np.float32).reshape(S, D)
    kTm = np.ascontiguousarray(np.asarray(keys, np.float32).reshape(S, D).T
                               ).astype(ml_dtypes.bfloat16)
    vTm = np.ascontiguousarray(np.asarray(values, np.float32).reshape(S, D).T
                               ).astype(ml_dtypes.bfloat16)
    Wq = np.asarray(Wq, np.float32)
    Wk = np.asarray(Wk, np.float32)
    Wv = np.asarray(Wv, np.float32)
    Wo = np.asarray(Wo, np.float32)
    qT_half = [np.ascontiguousarray(q[sh * CHQ:(sh + 1) * CHQ].T
                                    ).astype(ml_dtypes.bfloat16)
               for sh in range(2)]
    in_maps = []
    for c in range(NCORES):
        p, sh = c // 2, c % 2
        sl = slice(128 * p, 128 * p + 128)
        Wo2 = np.ascontiguousarray(Wo[sl, :] @ Wo).astype(ml_dtypes.bfloat16)
        in_maps.append({
            "qT": qT_half[sh], "kT": kTm, "vT": vTm,
            "Wq": np.ascontiguousarray(Wq[:, sl]).astype(ml_dtypes.bfloat16),
            "Wk": np.ascontiguousarray(Wk[:, sl]).astype(ml_dtypes.bfloat16),
            "Wv": np.ascontiguousarray(Wv[:, sl]).astype(ml_dtypes.bfloat16),
            "Wo2": Wo2,
        })
    return in_maps


def _run(in_maps, **kw):
    nc = _get_program()
    return run_bass_kernel_spmd(nc, in_maps, core_ids=list(range(NCORES)), **kw)


def _assemble(res, Wo, bo):
    bo2 = np.asarray(bo, np.float32) @ np.asarray(Wo, np.float32) \
        + np.asarray(bo, np.float32)
    halves = []
    for sh in range(2):
        acc = np.zeros((D, CHQ), np.float32)
        for p in range(4):
            acc += res.results[2 * p + sh]["yT"]
        halves.append(acc.T + bo2)
    return np.concatenate(halves, axis=0).reshape(1, S, D)


def kernel(queries, keys, values, Wq, Wk, Wv, Wo, bo):
    res = _run(_make_in_maps(queries, keys, values, Wq, Wk, Wv, Wo, bo))
    return _assemble(res, Wo, bo)


def run_traced(queries, keys, values, Wq, Wk, Wv, Wo, bo):
    """Like kernel() but with NTFF profiling; returns (output, BassKernelResults)."""
    import types
    import trn_agent_boot.trn_boot as _tb
    from concourse import bass_utils
    hook = _tb._ntff_profile_via_ctypes("/opt/axon/libaxon_pjrt.so")
    mod = types.ModuleType("antenv.axon_hooks")
    mod.get_axon_ntff_profile_hook = lambda: hook
    sys.modules["antenv.axon_hooks"] = mod
    bass_utils.upload_artifacts = lambda tmpdir: tmpdir
    res = _run(_make_in_maps(queries, keys, values, Wq, Wk, Wv, Wo, bo), trace=True)
    return _assemble(res, Wo, bo), res
